# revision 38
# baseline (speedup 1.0000x reference)
"""EncNet vq_codebook kernel for 8 Trainium2 NeuronCores.

Math (per reference):
  xs = x[:, :, 0, :].T                         # (b, s, c)
  d2[s,k]   = x2[s] - 2*cross[s,k] + cw2[k]
  a         = softmax_k(sm[k] * d2)
  e[b,k,c]  = sum_s a*xs - (sum_s a)*cw[k,c]
  BN over (b,c) (training stats), relu, mean over k, fc, sigmoid
  out = x * scale[b,c]

Distribution: data-parallel over batch (2 batches per core); BN batch
stats all-reduced across the 8 cores as a (64,2) tensor.

On-core layout: s-chunks of 128 land on PSUM partitions.  With an
x-chunk (c=128, s=128) as PE weights:
  - rhs = I                  -> xT chunk (s, c)     (transpose for free)
  - rhs = -2*sm_k*cw[k,c]    -> -2*sm_k*cross[s,k]
and with x^2 (fp16) as weights:
  - rhs = smhi (fp16)        -> sm_k * x2[s]        (fp16 sm; the per-k
    systematic logit error BN's per-k affine cancels, the rest is tiny)
so PSUM accumulates L[s,k] = sm_k*(x2[s] - 2cross[s,k]).  The constant
exp(sm_k*cw2_k) factor multiplies into araw on the DVE after the exp
(replicated row broadcast), so no PSUM seed matmul is needed; logits
are <= 0 by construction so exp without max-subtraction is safe.

Eight s-subchunks share PSUM banks per group so the softmax
element-wise work runs as (128,512) ops, not (128,64) ones (per-op
overhead dominates otherwise).  The transpose matmul emits bf16
directly (is_transpose), halving its PSUM footprint and read cost.
BN stats are AllGathered (single mesh phase) and reduced locally;
phase 2 multiplies the scale into the resident x tile in place.
"""

import sys

import numpy as np

try:
    import concourse.bass as bass  # noqa: F401
except ImportError:
    sys.path.insert(0, "/opt/trn_rl_repo")

import concourse.bacc as bacc
import concourse.bass as bass
import concourse.mybir as mybir
import concourse.tile as tile
from concourse.bass_utils import run_bass_kernel_spmd
from concourse._compat import get_trn_type
from ml_dtypes import bfloat16
float16 = np.float16

F32 = mybir.dt.float32
BF16 = mybir.dt.bfloat16
FP16 = mybir.dt.float16
ALU = mybir.AluOpType
ACTF = mybir.ActivationFunctionType

N_CORES = 8
B, C, SEQ, K = 16, 128, 16384, 64
B_LOC = B // N_CORES           # 2 batches per core
BIG = 2048                     # DMA chunk (free dim)
GRP = 1024                     # softmax group: 8 subchunks share PSUM banks
SUB = 128                      # s-subchunk = PSUM partition dim
BN_EPS = 1e-5


def build_program(seq=SEQ, b_loc=B_LOC, n_cores=N_CORES, big=BIG):
    n_big = seq // big
    n_grp = big // GRP
    n_sub = GRP // SUB         # 8

    nc = bacc.Bacc(
        get_trn_type() or "TRN2",
        target_bir_lowering=False,
        debug=False,
        num_devices=n_cores,
    )

    x_ap = nc.dram_tensor("x", [b_loc, C, seq], F32, kind="ExternalInput").ap()
    out_ap = nc.dram_tensor("out", [b_loc, C, seq], F32, kind="ExternalOutput").ap()

    def const_in(name, shape, dt):
        return nc.dram_tensor(name, shape, dt, kind="ExternalInput").ap()

    # constants packed into 4 DMAs (each dma_start costs ~0.65us of
    # sync-sequencer descriptor issue, delaying the x stream at launch)
    pkbf_d = const_in("pk_bf", [C, C + 2 * K], BF16)     # ident|cwt_sm|qrow
    smhi_d = const_in("smhi_fp16", [C, K], FP16)
    pkf32_d = const_in("pk_f32", [C, 2 * C], F32)        # cw_rows(pad)|fc_wt
    pkcol_d = const_in("pk_col", [C, 4], F32)            # gamma|beta|invk|fc_b

    with tile.TileContext(nc) as tc:
        with (
            tc.tile_pool(name="consts", bufs=1) as cpool,
            tc.tile_pool(name="xg", bufs=1) as xgp,
            tc.tile_pool(name="xsq", bufs=2) as xsqp,
            tc.tile_pool(name="xbf", bufs=2) as xbfp,
            tc.tile_pool(name="soft", bufs=4) as softp,
            tc.tile_pool(name="cols", bufs=8) as colp,
            tc.tile_pool(name="xt", bufs=4) as xtp,
            tc.tile_pool(name="etail", bufs=4) as etailp,
            tc.tile_pool(name="eloc", bufs=2) as elocp,
            tc.tile_pool(name="scales", bufs=2) as scalep,
            tc.tile_pool(name="ps_xt", bufs=3, space="PSUM") as ps_xt,
            tc.tile_pool(name="ps_L", bufs=4, space="PSUM") as ps_L,
            tc.tile_pool(name="ps_e", bufs=1, space="PSUM") as ps_e,
            tc.tile_pool(name="dram", bufs=2, space="DRAM") as dram,
        ):
            # ---- phase 1 chunking: small first chunk (PE starts sooner),
            # then 4096-wide chunks (16KB DMA descriptors, better HBM BW)
            sizes = [GRP, 2 * GRP, 4 * GRP, 4 * GRP, 4 * GRP, GRP]
            assert sum(sizes) == seq
            chunks = []
            pos = 0
            for sz in sizes:
                chunks.append((pos, sz))
                pos += sz

            # the first x chunk's DMA is issued before the const loads so
            # compute can begin as soon as the consts land
            xgs = [xgp.tile([C, seq], F32, name=f"xg{b}") for b in range(b_loc)]
            nc.sync.dma_start(
                out=xgs[0][:, 0 : chunks[0][1]], in_=x_ap[0, :, 0 : chunks[0][1]]
            )

            # ---- load constants into SBUF ----
            def load_const(dram_ap, shape, dt):
                t = cpool.tile(shape, dt, tag=dram_ap.tensor.name)
                nc.sync.dma_start(out=t[:], in_=dram_ap[:])
                return t

            pkbf = load_const(pkbf_d, [C, C + 2 * K], BF16)
            ident = pkbf[:, 0:C]
            cwt_sm = pkbf[:, C : C + K]
            qrow = pkbf[:, C + K : C + 2 * K]
            smhi_t = load_const(smhi_d, [C, K], FP16)
            smhi = smhi_t[:]
            pkf32 = load_const(pkf32_d, [C, 2 * C], F32)
            cw_rows = pkf32[0:K, 0:C]
            fc_wt = pkf32[:, C : 2 * C]
            pkcol = load_const(pkcol_d, [C, 4], F32)
            gamma = pkcol[0:K, 0:1]
            beta = pkcol[0:K, 1:2]
            invk = pkcol[0:K, 2:3]
            fc_b = pkcol[:, 3:4]

            # warm-up collective: pre-arm the CC engine mesh path while
            # phase 1 runs (nothing consumes the result)
            ccw_in = dram.tile([1, 2], F32, name="ccw_in")
            ccw_out = dram.tile([1, 2], F32, name="ccw_out")
            warm = colp.tile([1, 2], F32, tag="warm", name="warm")
            nc.vector.memset(warm[:], 0.0)
            nc.sync.dma_start(out=ccw_in[:], in_=warm[:])
            nc.gpsimd.collective_compute(
                "AllReduce",
                ALU.add,
                replica_groups=[list(range(n_cores))],
                ins=[ccw_in.opt()],
                outs=[ccw_out.opt()],
            )

            # ---- phase 1: per-batch aggregation e1|asum ----
            # x stays resident in SBUF for the whole run (used again by the
            # phase-2 scale), so HBM traffic is one read + one write of x.
            e_sbs = []
            xres = []
            for b in range(b_loc):
                e_ps = ps_e.tile([K, C + 1], F32)
                e_first = True
                xg = xgs[b]
                xres.append(xg)
                for j, (c0, csz) in enumerate(chunks):
                    jsl = slice(c0, c0 + csz)
                    if not (b == 0 and j == 0):
                        nc.sync.dma_start(out=xg[:, jsl], in_=x_ap[b, :, jsl])
                    xbf = xbfp.tile([C, csz], BF16, tag="xbf")
                    nc.scalar.copy(xbf[:], xg[:, jsl])
                    xsq = xsqp.tile([C, csz], FP16, tag="xsq")
                    nc.scalar.square(xsq[:], xg[:, jsl])
                    for g in range(csz // GRP):
                        g0 = g * GRP
                        # bf16 transpose output: whole group fits one PSUM
                        # bank and the DVE reads it in 2x mode
                        xt_ps = ps_xt.tile([SUB, n_sub * C], BF16)
                        L_ps = ps_L.tile([SUB, n_sub * K], F32, tag="L_ps")
                        for i in range(n_sub):
                            sl = slice(g0 + i * SUB, g0 + (i + 1) * SUB)
                            nc.tensor.matmul(
                                xt_ps[:, i * C : (i + 1) * C],
                                lhsT=xbf[:, sl], rhs=ident,
                                start=(i == 0), stop=(i == n_sub - 1),
                                is_transpose=True,
                                skip_group_check=True,
                            )
                            # i==0 start=True clears the whole L bank; later
                            # slots overwrite (accumulate bit still unset)
                            nc.tensor.matmul(
                                L_ps[:, i * K : (i + 1) * K],
                                lhsT=xbf[:, sl], rhs=cwt_sm,
                                start=(i == 0), stop=False,
                                skip_group_check=True,
                            )
                            nc.tensor.matmul(
                                L_ps[:, i * K : (i + 1) * K],
                                lhsT=xsq[:, sl], rhs=smhi,
                                start=False, stop=(i == n_sub - 1),
                                skip_group_check=True,
                            )
                        # araw = exp(sm_k*(x2-2cross)) * exp(sm_k*cw2_k);
                        # the constant factor multiplies in on the DVE
                        araw0 = softp.tile([SUB, n_sub * K], BF16, tag="araw0")
                        nc.scalar.activation(araw0[:], L_ps[:], ACTF.Exp)
                        araw = softp.tile([SUB, n_sub * K], BF16, tag="araw")
                        nc.vector.tensor_tensor(
                            araw[:].rearrange("p (g k) -> p g k", g=n_sub),
                            araw0[:].rearrange("p (g k) -> p g k", g=n_sub),
                            qrow.unsqueeze(1).broadcast_to(
                                [SUB, n_sub, K]
                            ),
                            ALU.mult,
                        )
                        zw = colp.tile([SUB, n_sub], F32, tag="zw")
                        nc.vector.tensor_reduce(
                            zw[:],
                            araw[:].rearrange("p (g k) -> p g k", g=n_sub),
                            mybir.AxisListType.X, ALU.add,
                        )
                        rz = colp.tile([SUB, n_sub], F32, tag="rz")
                        nc.vector.reciprocal(rz[:], zw[:])
                        rz_bf = colp.tile([SUB, n_sub], BF16, tag="rz_bf")
                        nc.vector.tensor_copy(rz_bf[:], rz[:])
                        # contiguous all-bf16 xtn so the DVE runs in 2x mode
                        # (a 129-pitch strided write defeats it); asum rides
                        # a separate tiny matmul instead
                        xtn = xtp.tile([SUB, n_sub * C], BF16)
                        nc.vector.tensor_tensor(
                            xtn[:].rearrange("p (g c) -> p g c", g=n_sub),
                            xt_ps[:].rearrange("p (g c) -> p g c", g=n_sub),
                            rz_bf[:].broadcast_to([SUB, n_sub, C]),
                            ALU.mult,
                        )
                        for i in range(n_sub):
                            last = (
                                j == len(chunks) - 1
                                and g == csz // GRP - 1
                                and i == n_sub - 1
                            )
                            nc.tensor.matmul(
                                e_ps[:, 0:C],
                                lhsT=araw[:, i * K : (i + 1) * K],
                                rhs=xtn[:, i * C : (i + 1) * C],
                                start=e_first, stop=last, skip_group_check=True,
                            )
                            nc.tensor.matmul(
                                e_ps[:, C : C + 1],
                                lhsT=araw[:, i * K : (i + 1) * K],
                                rhs=rz_bf[:, i : i + 1],
                                start=e_first, stop=last, skip_group_check=True,
                            )
                            e_first = False
                e_sb = etailp.tile([K, C + 1], F32, tag="e_sb")
                nc.vector.tensor_copy(e_sb[:], e_ps[:])
                e_sbs.append(e_sb)

            # ---- local e + stats ----
            e_locs, bstats = [], []
            for b in range(b_loc):
                e_sb = e_sbs[b]
                easm = etailp.tile([K, C], F32, tag="easm")
                nc.vector.tensor_scalar(
                    out=easm[:], in0=cw_rows, scalar1=e_sb[:, C : C + 1],
                    scalar2=None, op0=ALU.mult,
                )
                e_loc = elocp.tile([K, C], F32)
                nc.vector.tensor_tensor(e_loc[:], e_sb[:, 0:C], easm[:], ALU.subtract)
                e_locs.append(e_loc)
                bs = etailp.tile([K, 2], F32, tag=f"bs{b}")
                nc.vector.tensor_reduce(
                    bs[:, 0:1], e_loc[:], mybir.AxisListType.X, ALU.add
                )
                esq = etailp.tile([K, C], F32, tag="esq")
                nc.vector.scalar_tensor_tensor(
                    out=esq[:], in0=e_loc[:], scalar=1.0, in1=e_loc[:],
                    op0=ALU.mult, op1=ALU.mult, accum_out=bs[:, 1:2],
                )
                bstats.append(bs)

            stats = etailp.tile([K, 2], F32, tag="stats")
            nc.vector.tensor_tensor(stats[:], bstats[0][:], bstats[1][:], ALU.add)

            # ---- all-gather BN stats across cores (single-phase mesh,
            # cheaper than AllReduce's reduce+gather), local reduce on DVE
            cc_in = dram.tile([K, 2], F32)
            cc_out = dram.tile([n_cores, K, 2], F32)
            nc.sync.dma_start(out=cc_in[:], in_=stats[:])
            nc.gpsimd.collective_compute(
                "AllGather",
                ALU.bypass,
                replica_groups=[list(range(n_cores))],
                ins=[cc_in.opt()],
                outs=[cc_out.opt()],
            )
            gall = etailp.tile([K, n_cores * 2], F32, tag="gall")
            nc.sync.dma_start(
                out=gall[:].rearrange("k (r t) -> k r t", r=n_cores),
                in_=cc_out[:].rearrange("r k t -> k r t"),
            )
            gst = etailp.tile([K, 2], F32, tag="gst")
            nc.vector.tensor_reduce(
                gst[:].rearrange("p (o t) -> p o t", o=1),
                gall[:].rearrange("p (r t) -> p t r", r=n_cores),
                mybir.AxisListType.X, ALU.add,
            )

            # ---- BN affine + relu + mean_k + fc + sigmoid (tiny) ----
            n_tot = float(B * C)  # stats population: all b, all c
            # mst = [mean | E[x^2]] in one op
            mst = etailp.tile([K, 2], F32, tag="mst")
            nc.vector.tensor_scalar(
                out=mst[:], in0=gst[:], scalar1=1.0 / n_tot, scalar2=None,
                op0=ALU.mult,
            )
            mean = mst[:, 0:1]
            # nvar = mean^2 - E[x^2] = -var in one stt op; the sqrt then
            # applies scale=-1 bias=eps: stdv = sqrt(var + eps)
            varep = colp.tile([K, 1], F32, tag="varep")
            nc.vector.scalar_tensor_tensor(
                out=varep[:], in0=mean, scalar=mean, in1=mst[:, 1:2],
                op0=ALU.mult, op1=ALU.subtract,
            )
            eps_col = colp.tile([K, 1], F32, tag="eps_col")
            nc.vector.memset(eps_col[:], BN_EPS)
            stdv = colp.tile([K, 1], F32, tag="stdv")
            nc.scalar.activation(
                stdv[:], varep[:], ACTF.Sqrt, bias=eps_col[:], scale=-1.0
            )
            rstd = colp.tile([K, 1], F32, tag="rstd")
            nc.vector.reciprocal(rstd[:], stdv[:])
            psc = colp.tile([K, 1], F32, tag="psc")
            nc.vector.tensor_tensor(psc[:], gamma, rstd[:], ALU.mult)
            # pofs = beta - mean*psc
            pofs = colp.tile([K, 1], F32, tag="pofs")
            nc.vector.scalar_tensor_tensor(
                out=pofs[:], in0=mean, scalar=psc[:], in1=beta,
                op0=ALU.mult, op1=ALU.subtract,
            )
            nc.vector.tensor_scalar(
                out=pofs[:], in0=pofs[:], scalar1=-1.0, scalar2=None,
                op0=ALU.mult,
            )

            scale_cols = []
            for b in range(b_loc):
                reb = etailp.tile([K, C], F32, tag="reb")
                nc.scalar.activation(
                    reb[:], e_locs[b][:], ACTF.Relu, bias=pofs[:], scale=psc[:]
                )
                en_ps = ps_L.tile([C, 1], F32, tag="L_ps")
                nc.tensor.matmul(
                    en_ps[:], lhsT=reb[:], rhs=invk, start=True, stop=True
                )
                en_sb = colp.tile([C, 1], F32, tag="en_sb")
                nc.vector.tensor_copy(en_sb[:], en_ps[:])
                fc_ps = ps_L.tile([C, 1], F32, tag="L_ps")
                nc.tensor.matmul(
                    fc_ps[:], lhsT=fc_wt, rhs=en_sb[:], start=True, stop=True
                )
                sc = scalep.tile([C, 1], F32)
                nc.scalar.activation(sc[:], fc_ps[:], ACTF.Sigmoid, bias=fc_b)
                scale_cols.append(sc)

            # ---- phase 2: out = x * scale, written in place over the
            # resident x tile (no extra SBUF, no copy)
            for b in range(b_loc):
                for c0, csz in chunks:
                    jsl = slice(c0, c0 + csz)
                    nc.vector.tensor_scalar(
                        out=xres[b][:, jsl], in0=xres[b][:, jsl],
                        scalar1=scale_cols[b][:], scalar2=None, op0=ALU.mult,
                    )
                    nc.sync.dma_start(out=out_ap[b, :, jsl], in_=xres[b][:, jsl])

    nc.compile()
    return nc


def make_const_inputs(codewords, smoothing, bn_weight, bn_bias, fc_w, fc_b):
    cw = np.asarray(codewords, np.float32)        # (K, C)
    sm = np.asarray(smoothing, np.float32)        # (K,)
    cw2 = (cw * cw).sum(1)                        # (K,)
    smhi = sm.astype(float16)
    n_sub = GRP // SUB
    pk_bf = np.zeros((C, C + 2 * K), dtype=bfloat16)
    pk_bf[:, 0:C] = np.eye(C, dtype=bfloat16)
    pk_bf[:, C : C + K] = (cw.T * (-2.0 * sm)[None, :]).astype(bfloat16)
    pk_bf[:, C + K : C + 2 * K] = np.tile(
        np.exp(sm * cw2)[None, :], (SUB, 1)
    ).astype(bfloat16)
    pk_f32 = np.zeros((C, 2 * C), dtype=np.float32)
    pk_f32[0:K, 0:C] = cw
    pk_f32[:, C : 2 * C] = np.asarray(fc_w, np.float32).T  # (C_in, C_out)
    pk_col = np.zeros((C, 4), dtype=np.float32)
    pk_col[0:K, 0] = np.asarray(bn_weight, np.float32)
    pk_col[0:K, 1] = np.asarray(bn_bias, np.float32)
    pk_col[0:K, 2] = 1.0 / K
    pk_col[:, 3] = np.asarray(fc_b, np.float32)
    consts = {
        "pk_bf": pk_bf,
        "smhi_fp16": np.tile(smhi[None, :], (C, 1)),
        "pk_f32": pk_f32,
        "pk_col": pk_col,
    }
    return consts


_NC_CACHE = {}


def _get_program():
    key = (SEQ, B_LOC, N_CORES, BIG)
    if key not in _NC_CACHE:
        _NC_CACHE[key] = build_program(*key)
    return _NC_CACHE[key]


def _run(inputs, trace=False, trace_kwargs=None):
    x = np.asarray(inputs["x"], np.float32)
    assert x.shape == (B, C, 1, SEQ), x.shape
    xs = np.ascontiguousarray(x.reshape(B, C, SEQ))
    consts = make_const_inputs(
        inputs["codewords"], inputs["smoothing"], inputs["bn_weight"],
        inputs["bn_bias"], inputs["fc_w"], inputs["fc_b"],
    )
    in_maps = [
        {"x": np.ascontiguousarray(xs[i * B_LOC : (i + 1) * B_LOC]), **consts}
        for i in range(N_CORES)
    ]
    nc = _get_program()
    res = run_bass_kernel_spmd(
        nc, in_maps, core_ids=list(range(N_CORES)), trace=trace,
        **(trace_kwargs or {}),
    )
    out = np.concatenate([res.results[i]["out"] for i in range(N_CORES)], axis=0)
    return out.reshape(B, C, 1, SEQ).astype(np.float32), res


def kernel(**inputs):
    out, _ = _run(inputs)
    return out



# revision 39
# speedup vs baseline: 1.0903x; 1.0903x over previous
"""EncNet vq_codebook kernel for 8 Trainium2 NeuronCores.

Math (per reference):
  xs = x[:, :, 0, :].T                         # (b, s, c)
  d2[s,k]   = x2[s] - 2*cross[s,k] + cw2[k]
  a         = softmax_k(sm[k] * d2)
  e[b,k,c]  = sum_s a*xs - (sum_s a)*cw[k,c]
  BN over (b,c) (training stats), relu, mean over k, fc, sigmoid
  out = x * scale[b,c]

Distribution: data-parallel over batch (2 batches per core); BN batch
stats all-reduced across the 8 cores as a (64,2) tensor.

On-core layout: s-chunks of 128 land on PSUM partitions.  With an
x-chunk (c=128, s=128) as PE weights:
  - rhs = I                  -> xT chunk (s, c)     (transpose for free)
  - rhs = -2*sm_k*cw[k,c]    -> -2*sm_k*cross[s,k]
and with x^2 (fp16) as weights:
  - rhs = smhi (fp16)        -> sm_k * x2[s]        (fp16 sm; the per-k
    systematic logit error BN's per-k affine cancels, the rest is tiny)
so PSUM accumulates L[s,k] = sm_k*(x2[s] - 2cross[s,k]).  The constant
exp(sm_k*cw2_k) factor multiplies into araw on the DVE after the exp
(replicated row broadcast), so no PSUM seed matmul is needed; logits
are <= 0 by construction so exp without max-subtraction is safe.

Eight s-subchunks share PSUM banks per group so the softmax
element-wise work runs as (128,512) ops, not (128,64) ones (per-op
overhead dominates otherwise).  The transpose matmul emits bf16
directly (is_transpose), halving its PSUM footprint and read cost.
BN stats are AllGathered (single mesh phase) and reduced locally;
phase 2 multiplies the scale into the resident x tile in place.
"""

import sys

import numpy as np

try:
    import concourse.bass as bass  # noqa: F401
except ImportError:
    sys.path.insert(0, "/opt/trn_rl_repo")

import concourse.bacc as bacc
import concourse.bass as bass
import concourse.mybir as mybir
import concourse.tile as tile
from concourse.bass_utils import run_bass_kernel_spmd
from concourse._compat import get_trn_type
from ml_dtypes import bfloat16
float16 = np.float16

F32 = mybir.dt.float32
BF16 = mybir.dt.bfloat16
FP16 = mybir.dt.float16
ALU = mybir.AluOpType
ACTF = mybir.ActivationFunctionType

N_CORES = 8
B, C, SEQ, K = 16, 128, 16384, 64
B_LOC = B // N_CORES           # 2 batches per core
BIG = 2048                     # DMA chunk (free dim)
GRP = 1024                     # softmax group: 8 subchunks share PSUM banks
SUB = 128                      # s-subchunk = PSUM partition dim
BN_EPS = 1e-5


def build_program(seq=SEQ, b_loc=B_LOC, n_cores=N_CORES, big=BIG):
    n_big = seq // big
    n_grp = big // GRP
    n_sub = GRP // SUB         # 8

    nc = bacc.Bacc(
        get_trn_type() or "TRN2",
        target_bir_lowering=False,
        debug=False,
        num_devices=n_cores,
    )

    x_ap = nc.dram_tensor("x", [b_loc, C, seq], F32, kind="ExternalInput").ap()
    out_ap = nc.dram_tensor("out", [b_loc, C, seq], F32, kind="ExternalOutput").ap()

    def const_in(name, shape, dt):
        return nc.dram_tensor(name, shape, dt, kind="ExternalInput").ap()

    # constants packed into 4 DMAs (each dma_start costs ~0.65us of
    # sync-sequencer descriptor issue, delaying the x stream at launch)
    pkbf_d = const_in("pk_bf", [C, C + 2 * K], BF16)     # ident|cwt_sm|qrow
    smhi_d = const_in("smhi_fp16", [C, K], FP16)
    pkf32_d = const_in("pk_f32", [C, 2 * C], F32)        # cw_rows(pad)|fc_wt
    pkcol_d = const_in("pk_col", [C, 4], F32)            # gamma|beta|invk|fc_b

    with tile.TileContext(nc) as tc:
        with (
            tc.tile_pool(name="consts", bufs=1) as cpool,
            tc.tile_pool(name="xg", bufs=1) as xgp,
            tc.tile_pool(name="xsq", bufs=2) as xsqp,
            tc.tile_pool(name="xbf", bufs=2) as xbfp,
            tc.tile_pool(name="soft", bufs=4) as softp,
            tc.tile_pool(name="cols", bufs=8) as colp,
            tc.tile_pool(name="xt", bufs=4) as xtp,
            tc.tile_pool(name="etail", bufs=4) as etailp,
            tc.tile_pool(name="eloc", bufs=2) as elocp,
            tc.tile_pool(name="scales", bufs=2) as scalep,
            tc.tile_pool(name="ps_xt", bufs=3, space="PSUM") as ps_xt,
            tc.tile_pool(name="ps_L", bufs=4, space="PSUM") as ps_L,
            tc.tile_pool(name="ps_e", bufs=1, space="PSUM") as ps_e,
            tc.tile_pool(name="dram", bufs=2, space="DRAM") as dram,
        ):
            # ---- phase 1 chunking: small first chunk (PE starts sooner),
            # then 4096-wide chunks (16KB DMA descriptors, better HBM BW)
            sizes = [GRP, 2 * GRP, 4 * GRP, 4 * GRP, 4 * GRP, GRP]
            assert sum(sizes) == seq
            chunks = []
            pos = 0
            for sz in sizes:
                chunks.append((pos, sz))
                pos += sz

            # the first x chunk's DMA is issued before the const loads so
            # compute can begin as soon as the consts land
            xgs = [xgp.tile([C, seq], F32, name=f"xg{b}") for b in range(b_loc)]
            nc.sync.dma_start(
                out=xgs[0][:, 0 : chunks[0][1]], in_=x_ap[0, :, 0 : chunks[0][1]]
            )

            # ---- load constants into SBUF ----
            def load_const(dram_ap, shape, dt):
                t = cpool.tile(shape, dt, tag=dram_ap.tensor.name)
                nc.sync.dma_start(out=t[:], in_=dram_ap[:])
                return t

            pkbf = load_const(pkbf_d, [C, C + 2 * K], BF16)
            ident = pkbf[:, 0:C]
            cwt_sm = pkbf[:, C : C + K]
            qrow = pkbf[:, C + K : C + 2 * K]
            smhi_t = load_const(smhi_d, [C, K], FP16)
            smhi = smhi_t[:]
            pkf32 = load_const(pkf32_d, [C, 2 * C], F32)
            cw_rows = pkf32[0:K, 0:C]
            fc_wt = pkf32[:, C : 2 * C]
            pkcol = load_const(pkcol_d, [C, 4], F32)
            gamma = pkcol[0:K, 0:1]
            beta = pkcol[0:K, 1:2]
            invk = pkcol[0:K, 2:3]
            fc_b = pkcol[:, 3:4]

            # warm-up collective: pre-arm the CC engine mesh path while
            # phase 1 runs (nothing consumes the result)
            ccw_in = dram.tile([1, 2], F32, name="ccw_in")
            ccw_out = dram.tile([1, 2], F32, name="ccw_out")
            warm = colp.tile([1, 2], F32, tag="warm", name="warm")
            nc.vector.memset(warm[:], 0.0)
            nc.sync.dma_start(out=ccw_in[:], in_=warm[:])
            nc.gpsimd.collective_compute(
                "AllReduce",
                ALU.add,
                replica_groups=[list(range(n_cores))],
                ins=[ccw_in.opt()],
                outs=[ccw_out.opt()],
            )

            # ---- phase 1: per-batch aggregation e1|asum ----
            # x stays resident in SBUF for the whole run (used again by the
            # phase-2 scale), so HBM traffic is one read + one write of x.
            e_sbs = []
            xres = []
            for b in range(b_loc):
                e_ps = ps_e.tile([K, C + 1], F32)
                e_first = True
                xg = xgs[b]
                xres.append(xg)
                for j, (c0, csz) in enumerate(chunks):
                    jsl = slice(c0, c0 + csz)
                    if not (b == 0 and j == 0):
                        nc.sync.dma_start(out=xg[:, jsl], in_=x_ap[b, :, jsl])
                    xbf = xbfp.tile([C, csz], BF16, tag="xbf")
                    nc.scalar.copy(xbf[:], xg[:, jsl])
                    xsq = xsqp.tile([C, csz], FP16, tag="xsq")
                    nc.scalar.square(xsq[:], xg[:, jsl])
                    for g in range(csz // GRP):
                        g0 = g * GRP
                        # bf16 transpose output: whole group fits one PSUM
                        # bank and the DVE reads it in 2x mode
                        xt_ps = ps_xt.tile([SUB, n_sub * C], BF16)
                        L_ps = ps_L.tile([SUB, n_sub * K], F32, tag="L_ps")
                        for i in range(n_sub):
                            sl = slice(g0 + i * SUB, g0 + (i + 1) * SUB)
                            nc.tensor.matmul(
                                xt_ps[:, i * C : (i + 1) * C],
                                lhsT=xbf[:, sl], rhs=ident,
                                start=(i == 0), stop=(i == n_sub - 1),
                                is_transpose=True,
                                skip_group_check=True,
                            )
                            # i==0 start=True clears the whole L bank; later
                            # slots overwrite (accumulate bit still unset)
                            nc.tensor.matmul(
                                L_ps[:, i * K : (i + 1) * K],
                                lhsT=xbf[:, sl], rhs=cwt_sm,
                                start=(i == 0), stop=False,
                                skip_group_check=True,
                            )
                            nc.tensor.matmul(
                                L_ps[:, i * K : (i + 1) * K],
                                lhsT=xsq[:, sl], rhs=smhi,
                                start=False, stop=(i == n_sub - 1),
                                skip_group_check=True,
                            )
                        # araw = exp(sm_k*(x2-2cross)) * exp(sm_k*cw2_k);
                        # the constant factor multiplies in on the DVE
                        araw0 = softp.tile([SUB, n_sub * K], BF16, tag="araw0")
                        nc.scalar.activation(araw0[:], L_ps[:], ACTF.Exp)
                        araw = softp.tile([SUB, n_sub * K], BF16, tag="araw")
                        nc.vector.tensor_tensor(
                            araw[:].rearrange("p (g k) -> p g k", g=n_sub),
                            araw0[:].rearrange("p (g k) -> p g k", g=n_sub),
                            qrow.unsqueeze(1).broadcast_to(
                                [SUB, n_sub, K]
                            ),
                            ALU.mult,
                        )
                        zw = colp.tile([SUB, n_sub], F32, tag="zw")
                        nc.vector.tensor_reduce(
                            zw[:],
                            araw[:].rearrange("p (g k) -> p g k", g=n_sub),
                            mybir.AxisListType.X, ALU.add,
                        )
                        rz = colp.tile([SUB, n_sub], F32, tag="rz")
                        nc.vector.reciprocal(rz[:], zw[:])
                        rz_bf = colp.tile([SUB, n_sub], BF16, tag="rz_bf")
                        nc.vector.tensor_copy(rz_bf[:], rz[:])
                        # contiguous all-bf16 xtn so the DVE runs in 2x mode
                        # (a 129-pitch strided write defeats it); asum rides
                        # a separate tiny matmul instead
                        xtn = xtp.tile([SUB, n_sub * C], BF16)
                        nc.vector.tensor_tensor(
                            xtn[:].rearrange("p (g c) -> p g c", g=n_sub),
                            xt_ps[:].rearrange("p (g c) -> p g c", g=n_sub),
                            rz_bf[:].broadcast_to([SUB, n_sub, C]),
                            ALU.mult,
                        )
                        for i in range(n_sub):
                            last = (
                                j == len(chunks) - 1
                                and g == csz // GRP - 1
                                and i == n_sub - 1
                            )
                            nc.tensor.matmul(
                                e_ps[:, 0:C],
                                lhsT=araw[:, i * K : (i + 1) * K],
                                rhs=xtn[:, i * C : (i + 1) * C],
                                start=e_first, stop=last, skip_group_check=True,
                            )
                            nc.tensor.matmul(
                                e_ps[:, C : C + 1],
                                lhsT=araw[:, i * K : (i + 1) * K],
                                rhs=rz_bf[:, i : i + 1],
                                start=False, stop=last, skip_group_check=True,
                            )
                            e_first = False
                e_sb = etailp.tile([K, C + 1], F32, tag="e_sb")
                nc.vector.tensor_copy(e_sb[:], e_ps[:])
                e_sbs.append(e_sb)

            # ---- local e + stats ----
            e_locs, bstats = [], []
            for b in range(b_loc):
                e_sb = e_sbs[b]
                easm = etailp.tile([K, C], F32, tag="easm")
                nc.vector.tensor_scalar(
                    out=easm[:], in0=cw_rows, scalar1=e_sb[:, C : C + 1],
                    scalar2=None, op0=ALU.mult,
                )
                e_loc = elocp.tile([K, C], F32)
                nc.vector.tensor_tensor(e_loc[:], e_sb[:, 0:C], easm[:], ALU.subtract)
                e_locs.append(e_loc)
                bs = etailp.tile([K, 2], F32, tag=f"bs{b}")
                nc.vector.tensor_reduce(
                    bs[:, 0:1], e_loc[:], mybir.AxisListType.X, ALU.add
                )
                esq = etailp.tile([K, C], F32, tag="esq")
                nc.vector.scalar_tensor_tensor(
                    out=esq[:], in0=e_loc[:], scalar=1.0, in1=e_loc[:],
                    op0=ALU.mult, op1=ALU.mult, accum_out=bs[:, 1:2],
                )
                bstats.append(bs)

            stats = etailp.tile([K, 2], F32, tag="stats")
            nc.vector.tensor_tensor(stats[:], bstats[0][:], bstats[1][:], ALU.add)

            # ---- all-gather BN stats across cores (single-phase mesh,
            # cheaper than AllReduce's reduce+gather), local reduce on DVE
            cc_in = dram.tile([K, 2], F32)
            cc_out = dram.tile([n_cores, K, 2], F32)
            nc.sync.dma_start(out=cc_in[:], in_=stats[:])
            nc.gpsimd.collective_compute(
                "AllGather",
                ALU.bypass,
                replica_groups=[list(range(n_cores))],
                ins=[cc_in.opt()],
                outs=[cc_out.opt()],
            )
            gall = etailp.tile([K, n_cores * 2], F32, tag="gall")
            nc.sync.dma_start(
                out=gall[:].rearrange("k (r t) -> k r t", r=n_cores),
                in_=cc_out[:].rearrange("r k t -> k r t"),
            )
            gst = etailp.tile([K, 2], F32, tag="gst")
            nc.vector.tensor_reduce(
                gst[:].rearrange("p (o t) -> p o t", o=1),
                gall[:].rearrange("p (r t) -> p t r", r=n_cores),
                mybir.AxisListType.X, ALU.add,
            )

            # ---- BN affine + relu + mean_k + fc + sigmoid (tiny) ----
            n_tot = float(B * C)  # stats population: all b, all c
            # mst = [mean | E[x^2]] in one op
            mst = etailp.tile([K, 2], F32, tag="mst")
            nc.vector.tensor_scalar(
                out=mst[:], in0=gst[:], scalar1=1.0 / n_tot, scalar2=None,
                op0=ALU.mult,
            )
            mean = mst[:, 0:1]
            # nvar = mean^2 - E[x^2] = -var in one stt op; the sqrt then
            # applies scale=-1 bias=eps: stdv = sqrt(var + eps)
            varep = colp.tile([K, 1], F32, tag="varep")
            nc.vector.scalar_tensor_tensor(
                out=varep[:], in0=mean, scalar=mean, in1=mst[:, 1:2],
                op0=ALU.mult, op1=ALU.subtract,
            )
            eps_col = colp.tile([K, 1], F32, tag="eps_col")
            nc.vector.memset(eps_col[:], BN_EPS)
            stdv = colp.tile([K, 1], F32, tag="stdv")
            nc.scalar.activation(
                stdv[:], varep[:], ACTF.Sqrt, bias=eps_col[:], scale=-1.0
            )
            rstd = colp.tile([K, 1], F32, tag="rstd")
            nc.vector.reciprocal(rstd[:], stdv[:])
            psc = colp.tile([K, 1], F32, tag="psc")
            nc.vector.tensor_tensor(psc[:], gamma, rstd[:], ALU.mult)
            # pofs = beta - mean*psc
            pofs = colp.tile([K, 1], F32, tag="pofs")
            nc.vector.scalar_tensor_tensor(
                out=pofs[:], in0=mean, scalar=psc[:], in1=beta,
                op0=ALU.mult, op1=ALU.subtract,
            )
            nc.vector.tensor_scalar(
                out=pofs[:], in0=pofs[:], scalar1=-1.0, scalar2=None,
                op0=ALU.mult,
            )

            scale_cols = []
            for b in range(b_loc):
                reb = etailp.tile([K, C], F32, tag="reb")
                nc.scalar.activation(
                    reb[:], e_locs[b][:], ACTF.Relu, bias=pofs[:], scale=psc[:]
                )
                en_ps = ps_L.tile([C, 1], F32, tag="L_ps")
                nc.tensor.matmul(
                    en_ps[:], lhsT=reb[:], rhs=invk, start=True, stop=True
                )
                en_sb = colp.tile([C, 1], F32, tag="en_sb")
                nc.vector.tensor_copy(en_sb[:], en_ps[:])
                fc_ps = ps_L.tile([C, 1], F32, tag="L_ps")
                nc.tensor.matmul(
                    fc_ps[:], lhsT=fc_wt, rhs=en_sb[:], start=True, stop=True
                )
                sc = scalep.tile([C, 1], F32)
                nc.scalar.activation(sc[:], fc_ps[:], ACTF.Sigmoid, bias=fc_b)
                scale_cols.append(sc)

            # ---- phase 2: out = x * scale, written in place over the
            # resident x tile (no extra SBUF, no copy)
            for b in range(b_loc):
                for c0, csz in chunks:
                    jsl = slice(c0, c0 + csz)
                    nc.vector.tensor_scalar(
                        out=xres[b][:, jsl], in0=xres[b][:, jsl],
                        scalar1=scale_cols[b][:], scalar2=None, op0=ALU.mult,
                    )
                    nc.sync.dma_start(out=out_ap[b, :, jsl], in_=xres[b][:, jsl])

    nc.compile()
    return nc


def make_const_inputs(codewords, smoothing, bn_weight, bn_bias, fc_w, fc_b):
    cw = np.asarray(codewords, np.float32)        # (K, C)
    sm = np.asarray(smoothing, np.float32)        # (K,)
    cw2 = (cw * cw).sum(1)                        # (K,)
    smhi = sm.astype(float16)
    n_sub = GRP // SUB
    pk_bf = np.zeros((C, C + 2 * K), dtype=bfloat16)
    pk_bf[:, 0:C] = np.eye(C, dtype=bfloat16)
    pk_bf[:, C : C + K] = (cw.T * (-2.0 * sm)[None, :]).astype(bfloat16)
    pk_bf[:, C + K : C + 2 * K] = np.tile(
        np.exp(sm * cw2)[None, :], (SUB, 1)
    ).astype(bfloat16)
    pk_f32 = np.zeros((C, 2 * C), dtype=np.float32)
    pk_f32[0:K, 0:C] = cw
    pk_f32[:, C : 2 * C] = np.asarray(fc_w, np.float32).T  # (C_in, C_out)
    pk_col = np.zeros((C, 4), dtype=np.float32)
    pk_col[0:K, 0] = np.asarray(bn_weight, np.float32)
    pk_col[0:K, 1] = np.asarray(bn_bias, np.float32)
    pk_col[0:K, 2] = 1.0 / K
    pk_col[:, 3] = np.asarray(fc_b, np.float32)
    consts = {
        "pk_bf": pk_bf,
        "smhi_fp16": np.tile(smhi[None, :], (C, 1)),
        "pk_f32": pk_f32,
        "pk_col": pk_col,
    }
    return consts


_NC_CACHE = {}


def _get_program():
    key = (SEQ, B_LOC, N_CORES, BIG)
    if key not in _NC_CACHE:
        _NC_CACHE[key] = build_program(*key)
    return _NC_CACHE[key]


def _run(inputs, trace=False, trace_kwargs=None):
    x = np.asarray(inputs["x"], np.float32)
    assert x.shape == (B, C, 1, SEQ), x.shape
    xs = np.ascontiguousarray(x.reshape(B, C, SEQ))
    consts = make_const_inputs(
        inputs["codewords"], inputs["smoothing"], inputs["bn_weight"],
        inputs["bn_bias"], inputs["fc_w"], inputs["fc_b"],
    )
    in_maps = [
        {"x": np.ascontiguousarray(xs[i * B_LOC : (i + 1) * B_LOC]), **consts}
        for i in range(N_CORES)
    ]
    nc = _get_program()
    res = run_bass_kernel_spmd(
        nc, in_maps, core_ids=list(range(N_CORES)), trace=trace,
        **(trace_kwargs or {}),
    )
    out = np.concatenate([res.results[i]["out"] for i in range(N_CORES)], axis=0)
    return out.reshape(B, C, 1, SEQ).astype(np.float32), res


def kernel(**inputs):
    out, _ = _run(inputs)
    return out



# revision 40
# speedup vs baseline: 1.1082x; 1.0164x over previous
"""EncNet vq_codebook kernel for 8 Trainium2 NeuronCores.

Math (per reference):
  xs = x[:, :, 0, :].T                         # (b, s, c)
  d2[s,k]   = x2[s] - 2*cross[s,k] + cw2[k]
  a         = softmax_k(sm[k] * d2)
  e[b,k,c]  = sum_s a*xs - (sum_s a)*cw[k,c]
  BN over (b,c) (training stats), relu, mean over k, fc, sigmoid
  out = x * scale[b,c]

Distribution: data-parallel over batch (2 batches per core); BN batch
stats all-reduced across the 8 cores as a (64,2) tensor.

On-core layout: s-chunks of 128 land on PSUM partitions.  With an
x-chunk (c=128, s=128) as PE weights:
  - rhs = I                  -> xT chunk (s, c)     (transpose for free)
  - rhs = -2*sm_k*cw[k,c]    -> -2*sm_k*cross[s,k]
and with x^2 (fp16) as weights:
  - rhs = smhi (fp16)        -> sm_k * x2[s]        (fp16 sm; the per-k
    systematic logit error BN's per-k affine cancels, the rest is tiny)
so PSUM accumulates L[s,k] = sm_k*(x2[s] - 2cross[s,k]).  The constant
exp(sm_k*cw2_k) factor multiplies into araw on the DVE after the exp
(replicated row broadcast), so no PSUM seed matmul is needed; logits
are <= 0 by construction so exp without max-subtraction is safe.

Eight s-subchunks share PSUM banks per group so the softmax
element-wise work runs as (128,512) ops, not (128,64) ones (per-op
overhead dominates otherwise).  The transpose matmul emits bf16
directly (is_transpose), halving its PSUM footprint and read cost.
BN stats are AllGathered (single mesh phase) and reduced locally;
phase 2 multiplies the scale into the resident x tile in place.
"""

import sys

import numpy as np

try:
    import concourse.bass as bass  # noqa: F401
except ImportError:
    sys.path.insert(0, "/opt/trn_rl_repo")

import concourse.bacc as bacc
import concourse.bass as bass
import concourse.mybir as mybir
import concourse.tile as tile
from concourse.bass_utils import run_bass_kernel_spmd
from concourse._compat import get_trn_type
from ml_dtypes import bfloat16
float16 = np.float16

F32 = mybir.dt.float32
BF16 = mybir.dt.bfloat16
FP16 = mybir.dt.float16
ALU = mybir.AluOpType
ACTF = mybir.ActivationFunctionType

N_CORES = 8
B, C, SEQ, K = 16, 128, 16384, 64
B_LOC = B // N_CORES           # 2 batches per core
BIG = 2048                     # DMA chunk (free dim)
GRP = 1024                     # softmax group: 8 subchunks share PSUM banks
SUB = 128                      # s-subchunk = PSUM partition dim
BN_EPS = 1e-5


def build_program(seq=SEQ, b_loc=B_LOC, n_cores=N_CORES, big=BIG):
    n_big = seq // big
    n_grp = big // GRP
    n_sub = GRP // SUB         # 8

    nc = bacc.Bacc(
        get_trn_type() or "TRN2",
        target_bir_lowering=False,
        debug=False,
        num_devices=n_cores,
    )

    x_ap = nc.dram_tensor("x", [b_loc, C, seq], F32, kind="ExternalInput").ap()
    out_ap = nc.dram_tensor("out", [b_loc, C, seq], F32, kind="ExternalOutput").ap()

    def const_in(name, shape, dt):
        return nc.dram_tensor(name, shape, dt, kind="ExternalInput").ap()

    # constants packed into 4 DMAs (each dma_start costs ~0.65us of
    # sync-sequencer descriptor issue, delaying the x stream at launch)
    pkbf_d = const_in("pk_bf", [C, C + 2 * K], BF16)     # ident|cwt_sm|qrow
    smhi_d = const_in("smhi_fp16", [C, K], FP16)
    pkf32_d = const_in("pk_f32", [C, 2 * C], F32)        # cw_rows(pad)|fc_wt
    pkcol_d = const_in("pk_col", [C, 4], F32)            # gamma|beta|invk|fc_b

    with tile.TileContext(nc) as tc:
        with (
            tc.tile_pool(name="consts", bufs=1) as cpool,
            tc.tile_pool(name="xg", bufs=1) as xgp,
            tc.tile_pool(name="xsq", bufs=2) as xsqp,
            tc.tile_pool(name="xbf", bufs=2) as xbfp,
            tc.tile_pool(name="soft", bufs=4) as softp,
            tc.tile_pool(name="cols", bufs=8) as colp,
            tc.tile_pool(name="xt", bufs=4) as xtp,
            tc.tile_pool(name="etail", bufs=4) as etailp,
            tc.tile_pool(name="eloc", bufs=2) as elocp,
            tc.tile_pool(name="scales", bufs=2) as scalep,
            tc.tile_pool(name="ps_xt", bufs=3, space="PSUM") as ps_xt,
            tc.tile_pool(name="ps_L", bufs=4, space="PSUM") as ps_L,
            tc.tile_pool(name="ps_e", bufs=1, space="PSUM") as ps_e,
            tc.tile_pool(name="dram", bufs=2, space="DRAM") as dram,
        ):
            # ---- phase 1 chunking: small first chunk (PE starts sooner),
            # then 4096-wide chunks (16KB DMA descriptors, better HBM BW)
            sizes = [GRP, 2 * GRP, 4 * GRP, 4 * GRP, 4 * GRP, GRP]
            assert sum(sizes) == seq
            chunks = []
            pos = 0
            for sz in sizes:
                chunks.append((pos, sz))
                pos += sz

            # the first x chunk's DMA is issued before the const loads so
            # compute can begin as soon as the consts land
            xgs = [xgp.tile([C, seq], F32, name=f"xg{b}") for b in range(b_loc)]
            nc.sync.dma_start(
                out=xgs[0][:, 0 : chunks[0][1]], in_=x_ap[0, :, 0 : chunks[0][1]]
            )

            # ---- load constants into SBUF ----
            def load_const(dram_ap, shape, dt):
                t = cpool.tile(shape, dt, tag=dram_ap.tensor.name)
                nc.sync.dma_start(out=t[:], in_=dram_ap[:])
                return t

            pkbf = load_const(pkbf_d, [C, C + 2 * K], BF16)
            ident = pkbf[:, 0:C]
            cwt_sm = pkbf[:, C : C + K]
            qrow = pkbf[:, C + K : C + 2 * K]
            smhi_t = load_const(smhi_d, [C, K], FP16)
            smhi = smhi_t[:]
            pkf32 = load_const(pkf32_d, [C, 2 * C], F32)
            cw_rows = pkf32[0:K, 0:C]
            fc_wt = pkf32[:, C : 2 * C]
            pkcol = load_const(pkcol_d, [C, 4], F32)
            gamma = pkcol[0:K, 0:1]
            beta = pkcol[0:K, 1:2]
            invk = pkcol[0:K, 2:3]
            fc_b = pkcol[:, 3:4]

            # warm-up collective: pre-arm the CC engine mesh path while
            # phase 1 runs (nothing consumes the result)
            ccw_in = dram.tile([1, 2], F32, name="ccw_in")
            ccw_out = dram.tile([1, 2], F32, name="ccw_out")
            warm = colp.tile([1, 2], F32, tag="warm", name="warm")
            nc.vector.memset(warm[:], 0.0)
            nc.sync.dma_start(out=ccw_in[:], in_=warm[:])
            nc.gpsimd.collective_compute(
                "AllReduce",
                ALU.add,
                replica_groups=[list(range(n_cores))],
                ins=[ccw_in.opt()],
                outs=[ccw_out.opt()],
            )

            # ---- phase 1: per-batch aggregation e1|asum ----
            # x stays resident in SBUF for the whole run (used again by the
            # phase-2 scale), so HBM traffic is one read + one write of x.
            e_sbs = []
            xres = []
            for b in range(b_loc):
                e_ps = ps_e.tile([K, C + 1], F32)
                e_first = True
                xg = xgs[b]
                xres.append(xg)
                for j, (c0, csz) in enumerate(chunks):
                    jsl = slice(c0, c0 + csz)
                    if not (b == 0 and j == 0):
                        nc.sync.dma_start(out=xg[:, jsl], in_=x_ap[b, :, jsl])
                    xbf = xbfp.tile([C, csz], BF16, tag="xbf")
                    nc.scalar.copy(xbf[:], xg[:, jsl])
                    xsq = xsqp.tile([C, csz], FP16, tag="xsq")
                    nc.scalar.square(xsq[:], xg[:, jsl])
                    for g in range(csz // GRP):
                        g0 = g * GRP
                        # bf16 transpose output: whole group fits one PSUM
                        # bank and the DVE reads it in 2x mode
                        xt_ps = ps_xt.tile([SUB, n_sub * C], BF16)
                        L_ps = ps_L.tile([SUB, n_sub * K], F32, tag="L_ps")
                        for i in range(n_sub):
                            sl = slice(g0 + i * SUB, g0 + (i + 1) * SUB)
                            nc.tensor.matmul(
                                xt_ps[:, i * C : (i + 1) * C],
                                lhsT=xbf[:, sl], rhs=ident,
                                start=(i == 0), stop=(i == n_sub - 1),
                                is_transpose=True,
                                skip_group_check=True,
                            )
                            # i==0 start=True clears the whole L bank; later
                            # slots overwrite (accumulate bit still unset)
                            nc.tensor.matmul(
                                L_ps[:, i * K : (i + 1) * K],
                                lhsT=xbf[:, sl], rhs=cwt_sm,
                                start=(i == 0), stop=False,
                                skip_group_check=True,
                            )
                            nc.tensor.matmul(
                                L_ps[:, i * K : (i + 1) * K],
                                lhsT=xsq[:, sl], rhs=smhi,
                                start=False, stop=(i == n_sub - 1),
                                skip_group_check=True,
                            )
                        # araw = exp(sm_k*(x2-2cross)) * exp(sm_k*cw2_k);
                        # the constant factor multiplies in on the DVE
                        araw0 = softp.tile([SUB, n_sub * K], BF16, tag="araw0")
                        nc.scalar.activation(araw0[:], L_ps[:], ACTF.Exp)
                        araw = softp.tile([SUB, n_sub * K], BF16, tag="araw")
                        nc.vector.tensor_tensor(
                            araw[:].rearrange("p (g k) -> p g k", g=n_sub),
                            araw0[:].rearrange("p (g k) -> p g k", g=n_sub),
                            qrow.unsqueeze(1).broadcast_to(
                                [SUB, n_sub, K]
                            ),
                            ALU.mult,
                        )
                        zw = colp.tile([SUB, n_sub], F32, tag="zw")
                        nc.vector.tensor_reduce(
                            zw[:],
                            araw[:].rearrange("p (g k) -> p g k", g=n_sub),
                            mybir.AxisListType.X, ALU.add,
                        )
                        rz = colp.tile([SUB, n_sub], F32, tag="rz")
                        nc.vector.reciprocal(rz[:], zw[:])
                        rz_bf = colp.tile([SUB, n_sub], BF16, tag="rz_bf")
                        nc.vector.tensor_copy(rz_bf[:], rz[:])
                        # xtn packs [xt_i*rz | rz] per subchunk so the e and
                        # asum accumulations share one matmul (rhs 129 cols)
                        xtn = xtp.tile([SUB, n_sub * (C + 1)], BF16)
                        xtn3 = xtn[:].rearrange("p (g c) -> p g c", g=n_sub)
                        nc.vector.tensor_tensor(
                            xtn3[:, :, 0:C],
                            xt_ps[:].rearrange("p (g c) -> p g c", g=n_sub),
                            rz_bf[:].broadcast_to([SUB, n_sub, C]),
                            ALU.mult,
                        )
                        nc.vector.tensor_copy(
                            xtn3[:, :, C : C + 1], rz_bf[:].unsqueeze(2)
                        )
                        for i in range(n_sub):
                            last = (
                                j == len(chunks) - 1
                                and g == csz // GRP - 1
                                and i == n_sub - 1
                            )
                            nc.tensor.matmul(
                                e_ps[:, 0 : C + 1],
                                lhsT=araw[:, i * K : (i + 1) * K],
                                rhs=xtn[:, i * (C + 1) : (i + 1) * (C + 1)],
                                start=e_first, stop=last, skip_group_check=True,
                            )
                            e_first = False
                e_sb = etailp.tile([K, C + 1], F32, tag="e_sb")
                nc.vector.tensor_copy(e_sb[:], e_ps[:])
                e_sbs.append(e_sb)

            # ---- local e + stats ----
            e_locs, bstats = [], []
            for b in range(b_loc):
                e_sb = e_sbs[b]
                easm = etailp.tile([K, C], F32, tag="easm")
                nc.vector.tensor_scalar(
                    out=easm[:], in0=cw_rows, scalar1=e_sb[:, C : C + 1],
                    scalar2=None, op0=ALU.mult,
                )
                e_loc = elocp.tile([K, C], F32)
                nc.vector.tensor_tensor(e_loc[:], e_sb[:, 0:C], easm[:], ALU.subtract)
                e_locs.append(e_loc)
                bs = etailp.tile([K, 2], F32, tag=f"bs{b}")
                nc.vector.tensor_reduce(
                    bs[:, 0:1], e_loc[:], mybir.AxisListType.X, ALU.add
                )
                esq = etailp.tile([K, C], F32, tag="esq")
                nc.vector.scalar_tensor_tensor(
                    out=esq[:], in0=e_loc[:], scalar=1.0, in1=e_loc[:],
                    op0=ALU.mult, op1=ALU.mult, accum_out=bs[:, 1:2],
                )
                bstats.append(bs)

            stats = etailp.tile([K, 2], F32, tag="stats")
            nc.vector.tensor_tensor(stats[:], bstats[0][:], bstats[1][:], ALU.add)

            # ---- all-gather BN stats across cores (single-phase mesh,
            # cheaper than AllReduce's reduce+gather), local reduce on DVE
            cc_in = dram.tile([K, 2], F32)
            cc_out = dram.tile([n_cores, K, 2], F32)
            nc.sync.dma_start(out=cc_in[:], in_=stats[:])
            nc.gpsimd.collective_compute(
                "AllGather",
                ALU.bypass,
                replica_groups=[list(range(n_cores))],
                ins=[cc_in.opt()],
                outs=[cc_out.opt()],
            )
            gall = etailp.tile([K, n_cores * 2], F32, tag="gall")
            nc.sync.dma_start(
                out=gall[:].rearrange("k (r t) -> k r t", r=n_cores),
                in_=cc_out[:].rearrange("r k t -> k r t"),
            )
            gst = etailp.tile([K, 2], F32, tag="gst")
            nc.vector.tensor_reduce(
                gst[:].rearrange("p (o t) -> p o t", o=1),
                gall[:].rearrange("p (r t) -> p t r", r=n_cores),
                mybir.AxisListType.X, ALU.add,
            )

            # ---- BN affine + relu + mean_k + fc + sigmoid (tiny) ----
            n_tot = float(B * C)  # stats population: all b, all c
            # mst = [mean | E[x^2]] in one op
            mst = etailp.tile([K, 2], F32, tag="mst")
            nc.vector.tensor_scalar(
                out=mst[:], in0=gst[:], scalar1=1.0 / n_tot, scalar2=None,
                op0=ALU.mult,
            )
            mean = mst[:, 0:1]
            # nvar = mean^2 - E[x^2] = -var in one stt op; the sqrt then
            # applies scale=-1 bias=eps: stdv = sqrt(var + eps)
            varep = colp.tile([K, 1], F32, tag="varep")
            nc.vector.scalar_tensor_tensor(
                out=varep[:], in0=mean, scalar=mean, in1=mst[:, 1:2],
                op0=ALU.mult, op1=ALU.subtract,
            )
            eps_col = colp.tile([K, 1], F32, tag="eps_col")
            nc.vector.memset(eps_col[:], BN_EPS)
            stdv = colp.tile([K, 1], F32, tag="stdv")
            nc.scalar.activation(
                stdv[:], varep[:], ACTF.Sqrt, bias=eps_col[:], scale=-1.0
            )
            rstd = colp.tile([K, 1], F32, tag="rstd")
            nc.vector.reciprocal(rstd[:], stdv[:])
            psc = colp.tile([K, 1], F32, tag="psc")
            nc.vector.tensor_tensor(psc[:], gamma, rstd[:], ALU.mult)
            # pofs = beta - mean*psc
            pofs = colp.tile([K, 1], F32, tag="pofs")
            nc.vector.scalar_tensor_tensor(
                out=pofs[:], in0=mean, scalar=psc[:], in1=beta,
                op0=ALU.mult, op1=ALU.subtract,
            )
            nc.vector.tensor_scalar(
                out=pofs[:], in0=pofs[:], scalar1=-1.0, scalar2=None,
                op0=ALU.mult,
            )

            scale_cols = []
            for b in range(b_loc):
                reb = etailp.tile([K, C], F32, tag="reb")
                nc.scalar.activation(
                    reb[:], e_locs[b][:], ACTF.Relu, bias=pofs[:], scale=psc[:]
                )
                en_ps = ps_L.tile([C, 1], F32, tag="L_ps")
                nc.tensor.matmul(
                    en_ps[:], lhsT=reb[:], rhs=invk, start=True, stop=True
                )
                en_sb = colp.tile([C, 1], F32, tag="en_sb")
                nc.vector.tensor_copy(en_sb[:], en_ps[:])
                fc_ps = ps_L.tile([C, 1], F32, tag="L_ps")
                nc.tensor.matmul(
                    fc_ps[:], lhsT=fc_wt, rhs=en_sb[:], start=True, stop=True
                )
                sc = scalep.tile([C, 1], F32)
                nc.scalar.activation(sc[:], fc_ps[:], ACTF.Sigmoid, bias=fc_b)
                scale_cols.append(sc)

            # ---- phase 2: out = x * scale, written in place over the
            # resident x tile (no extra SBUF, no copy)
            for b in range(b_loc):
                for c0, csz in chunks:
                    jsl = slice(c0, c0 + csz)
                    nc.vector.tensor_scalar(
                        out=xres[b][:, jsl], in0=xres[b][:, jsl],
                        scalar1=scale_cols[b][:], scalar2=None, op0=ALU.mult,
                    )
                    nc.sync.dma_start(out=out_ap[b, :, jsl], in_=xres[b][:, jsl])

    nc.compile()
    return nc


def make_const_inputs(codewords, smoothing, bn_weight, bn_bias, fc_w, fc_b):
    cw = np.asarray(codewords, np.float32)        # (K, C)
    sm = np.asarray(smoothing, np.float32)        # (K,)
    cw2 = (cw * cw).sum(1)                        # (K,)
    smhi = sm.astype(float16)
    n_sub = GRP // SUB
    pk_bf = np.zeros((C, C + 2 * K), dtype=bfloat16)
    pk_bf[:, 0:C] = np.eye(C, dtype=bfloat16)
    pk_bf[:, C : C + K] = (cw.T * (-2.0 * sm)[None, :]).astype(bfloat16)
    pk_bf[:, C + K : C + 2 * K] = np.tile(
        np.exp(sm * cw2)[None, :], (SUB, 1)
    ).astype(bfloat16)
    pk_f32 = np.zeros((C, 2 * C), dtype=np.float32)
    pk_f32[0:K, 0:C] = cw
    pk_f32[:, C : 2 * C] = np.asarray(fc_w, np.float32).T  # (C_in, C_out)
    pk_col = np.zeros((C, 4), dtype=np.float32)
    pk_col[0:K, 0] = np.asarray(bn_weight, np.float32)
    pk_col[0:K, 1] = np.asarray(bn_bias, np.float32)
    pk_col[0:K, 2] = 1.0 / K
    pk_col[:, 3] = np.asarray(fc_b, np.float32)
    consts = {
        "pk_bf": pk_bf,
        "smhi_fp16": np.tile(smhi[None, :], (C, 1)),
        "pk_f32": pk_f32,
        "pk_col": pk_col,
    }
    return consts


_NC_CACHE = {}


def _get_program():
    key = (SEQ, B_LOC, N_CORES, BIG)
    if key not in _NC_CACHE:
        _NC_CACHE[key] = build_program(*key)
    return _NC_CACHE[key]


def _run(inputs, trace=False, trace_kwargs=None):
    x = np.asarray(inputs["x"], np.float32)
    assert x.shape == (B, C, 1, SEQ), x.shape
    xs = np.ascontiguousarray(x.reshape(B, C, SEQ))
    consts = make_const_inputs(
        inputs["codewords"], inputs["smoothing"], inputs["bn_weight"],
        inputs["bn_bias"], inputs["fc_w"], inputs["fc_b"],
    )
    in_maps = [
        {"x": np.ascontiguousarray(xs[i * B_LOC : (i + 1) * B_LOC]), **consts}
        for i in range(N_CORES)
    ]
    nc = _get_program()
    res = run_bass_kernel_spmd(
        nc, in_maps, core_ids=list(range(N_CORES)), trace=trace,
        **(trace_kwargs or {}),
    )
    out = np.concatenate([res.results[i]["out"] for i in range(N_CORES)], axis=0)
    return out.reshape(B, C, 1, SEQ).astype(np.float32), res


def kernel(**inputs):
    out, _ = _run(inputs)
    return out

